# revision 18
# baseline (speedup 1.0000x reference)
"""Multi-head attention (B=2, N=2048, EMB=1024, 16 heads) on 8 TRN2 NeuronCores.

Sharding: data-parallel over batch (2) x tensor-parallel over heads (4 groups of
4 heads).  Core c handles batch c//4 and heads [4*(c%4), 4*(c%4)+4).  Each core:
  - projects its batch's q/k/v against the column slice of Wq/Wk/Wv for its
    heads (activations pre-transposed on host to [EMB, N] so features sit on
    SBUF partitions),
  - runs attention for its 4 heads in S^T orientation ([k_tokens, q_tokens]):
    softmax row-sums come for free from a ones-column appended to V in the
    P@V matmul; exp runs on the scalar engine straight out of PSUM,
  - applies the row-parallel slice of Wo, producing a partial [EMB, N] output,
  - emits each head's normalized prob map (transposed, bf16) for the
    total_attention_weights diagnostic.
Host sums partial outputs across the 4 cores of each batch (+ bo) and sums the
16 per-head tw partials of batch-0 cores.  Device compute is bf16 with fp32
PSUM accumulation.

Program structure is ordered for engine overlap: the scalar engine's exp
stream is the steady-state bottleneck, so projection work for the second
head-pair and the output-projection matmuls of the previous q-chunk are
interleaved into the attention loops to fill the PE's slack, and the softmax
reciprocal chain (PSUM row -> SBUF -> packed DVE reciprocal -> broadcast
matmul) is deferred behind PE filler work so it never stalls the PE.
"""
import sys

sys.path.insert(0, "/opt/trn_rl_repo")

from contextlib import ExitStack

import numpy as np
import ml_dtypes

import concourse.bass as bass
import concourse.bacc as bacc
import concourse.tile as tile
from concourse import mybir
from concourse.bass_utils import run_bass_kernel_spmd

BF16 = mybir.dt.bfloat16
F32 = mybir.dt.float32
AF = mybir.ActivationFunctionType
NPBF = ml_dtypes.bfloat16

N = 2048          # sequence length
EMB = 1024        # model dim
HL = 4            # heads per core
D = 64            # head dim
EC = HL * D       # head dims per core (256)
KT = N // 128     # 16 k-token tiles
QW = 1024         # q chunk width
NQH = N // QW     # 2 q chunks
SCALE = 1.0 / np.sqrt(D)

_CACHE = {}


def build_program():
    nc = bacc.Bacc("TRN2", target_bir_lowering=False, debug=False, num_devices=8)

    xq_d = nc.declare_dram_parameter("xqT", [EMB, N], BF16, isOutput=False)
    xk_d = nc.declare_dram_parameter("xkT", [EMB, N], BF16, isOutput=False)
    xv_d = nc.declare_dram_parameter("xvT", [EMB, N], BF16, isOutput=False)
    wq_d = nc.declare_dram_parameter("wq", [EMB, EC], BF16, isOutput=False)
    wk_d = nc.declare_dram_parameter("wk", [EMB, EC], BF16, isOutput=False)
    wv_d = nc.declare_dram_parameter("wv", [EMB, EC], BF16, isOutput=False)
    wo_d = nc.declare_dram_parameter("wo", [EC, EMB], BF16, isOutput=False)
    bqf_d = nc.declare_dram_parameter("bqf", [128, 2], F32, isOutput=False)
    bkf_d = nc.declare_dram_parameter("bkf", [128, 2], F32, isOutput=False)
    bv_d = nc.declare_dram_parameter("bv", [1, EC], BF16, isOutput=False)
    out_d = nc.declare_dram_parameter("outT", [EMB, N], BF16, isOutput=True)
    tw_d = [nc.declare_dram_parameter(f"tw{h}", [N, N], BF16, isOutput=True)
            for h in range(HL)]

    with ExitStack() as ctx:
        tc = ctx.enter_context(tile.TileContext(nc))

        const = ctx.enter_context(tc.tile_pool(name="const", bufs=1))
        ones_bf = const.tile([1, 512], BF16, name="ones_bf")
        nc.vector.memset(ones_bf[:], 1.0)
        bqf_sb = const.tile([128, 2], F32, name="bqf_sb")
        bkf_sb = const.tile([128, 2], F32, name="bkf_sb")
        bv_sb = const.tile([1, EC], BF16, name="bv_sb")
        nc.sync.dma_start(bqf_sb[:], bqf_d[:])
        nc.sync.dma_start(bkf_sb[:], bkf_d[:])
        nc.sync.dma_start(bv_sb[:], bv_d[:])

        wpool = ctx.enter_context(tc.tile_pool(name="wpool", bufs=1))
        wq_sb = wpool.tile([128, 8, EC], BF16, name="wq_sb")  # k-chunk major
        wk_sb = wpool.tile([128, 8, EC], BF16, name="wk_sb")
        wv_sb = wpool.tile([128, 8, EC], BF16, name="wv_sb")
        wo_sb = wpool.tile([128, 2, EMB], BF16, name="wo_sb")
        for kc in range(8):
            nc.sync.dma_start(wk_sb[:, kc, :], wk_d[kc * 128:(kc + 1) * 128, :])
            nc.sync.dma_start(wq_sb[:, kc, :], wq_d[kc * 128:(kc + 1) * 128, :])
            nc.sync.dma_start(wv_sb[:, kc, :], wv_d[kc * 128:(kc + 1) * 128, :])
        for hc in range(2):
            nc.sync.dma_start(wo_sb[:, hc, :], wo_d[hc * 128:(hc + 1) * 128, :])

        qkv = ctx.enter_context(tc.tile_pool(name="qkv", bufs=1))
        qhT = qkv.tile([128, 2, N], BF16, name="qhT")      # [dim-half][128, tok]
        khT = qkv.tile([128, 2, N], BF16, name="khT")
        vh = qkv.tile([128, KT, HL, 65], BF16, name="vh")  # [tok128][kt][h][64+ones]
        an = qkv.tile([128, 2, N], BF16, name="an")        # A_norm^T [hd-chunk][q]
        nc.vector.memset(vh[:, :, :, 64:65], 1.0)

        # 16 slots: xk tiles are fully consumed by the upfront k-projections,
        # so the xv tiles recycle their slots; xq tiles stay live for the
        # deferred q-projection fillers.
        xpool = ctx.enter_context(tc.tile_pool(name="xpool", bufs=16))
        # Single shared PSUM pool for projection / scores / broadcast /
        # out-projection groups (tag "ps", 2 slots x 2 banks) + the attention
        # accumulators (4 banks) = all 8 PSUM banks.
        scps = ctx.enter_context(tc.tile_pool(name="scps", bufs=2, space="PSUM"))
        avps = ctx.enter_context(tc.tile_pool(name="avps", bufs=1, space="PSUM"))

        # ---- phase A: DMAs + projections (emitted piecewise for overlap) ----
        def load_x(x_d, pfx):
            xt = []
            for kc in range(8):
                xtile = xpool.tile([128, N], BF16, name=f"{pfx}{kc}", tag="x")
                nc.sync.dma_start(xtile[:], x_d[kc * 128:(kc + 1) * 128, :])
                xt.append(xtile)
            return xt

        xk_t = load_x(xk_d, "xk")
        xq_t = load_x(xq_d, "xq")
        xv_t = load_x(xv_d, "xv")

        def proj_qk_group(xt, w_sb, bf_sb, dst, dh, q4):
            ps = scps.tile([128, 512], F32, name="pj", tag="ps")
            for kc in range(8):
                nc.tensor.matmul(
                    ps[:], w_sb[:, kc, dh * 128:(dh + 1) * 128],
                    xt[kc][:, q4 * 512:(q4 + 1) * 512],
                    start=(kc == 0), stop=(kc == 7))
            nc.scalar.activation(dst[:, dh, q4 * 512:(q4 + 1) * 512], ps[:],
                                 AF.Identity, bias=bf_sb[:, dh:dh + 1], scale=1.0)

        def proj_v_group(t):
            ps = scps.tile([128, EC], F32, name="pv", tag="ps")
            for kc in range(8):
                nc.tensor.matmul(ps[:], xv_t[kc][:, t * 128:(t + 1) * 128],
                                 wv_sb[:, kc, :], start=(kc == 0), stop=False)
            nc.tensor.matmul(ps[:], ones_bf[0:1, 0:128], bv_sb[0:1, :],
                             start=False, stop=True)
            nc.scalar.copy(vh[:, t, :, 0:64],
                           ps[:].rearrange("p (h c) -> p h c", h=HL))

        # upfront: all of k (so the xk tiles can be recycled for xv), the qc0
        # window of q dim-half 0, and the first v tiles.
        for q4 in range(4):
            proj_qk_group(xk_t, wk_sb, bkf_sb, khT, 0, q4)
        for q4 in range(4):
            proj_qk_group(xk_t, wk_sb, bkf_sb, khT, 1, q4)
        for q4 in range(2):
            proj_qk_group(xq_t, wq_sb, bqf_sb, qhT, 0, q4)
        for t in range(8):
            proj_v_group(t)
        epool = ctx.enter_context(tc.tile_pool(name="epool", bufs=33))
        evp = ctx.enter_context(tc.tile_pool(name="evp", bufs=3))
        rbp = ctx.enter_context(tc.tile_pool(name="rbp", bufs=3))
        twp = ctx.enter_context(tc.tile_pool(name="twp", bufs=3))
        smol = ctx.enter_context(tc.tile_pool(name="smol", bufs=2))

        def outproj_group(qc, m, qn):
            """One output-projection PSUM group for q window qc (512 cols)."""
            q0 = qc * QW + qn * 512
            ps = scps.tile([128, 512], F32, name="op", tag="ps")
            for hc in range(2):
                nc.tensor.matmul(ps[:], wo_sb[:, hc, m * 128:(m + 1) * 128],
                                 an[:, hc, q0:q0 + 512],
                                 start=(hc == 0), stop=(hc == 1))
            ot = twp.tile([128, 512], BF16, name="ot", tag="tw")
            nc.vector.tensor_copy(ot[:], ps[:])
            nc.gpsimd.dma_start(
                out_d[m * 128:(m + 1) * 128, q0:q0 + 512], ot[:])

        # PE filler queue: closures emitting matmul-heavy groups, injected
        # into attention-loop slack so the PE never idles cold.  FIFO order
        # respects each deferred group's deadline (first use downstream).
        fillers = []

        def pop_fillers(k):
            for _ in range(min(k, len(fillers))):
                fillers.pop(0)()

        for q4 in range(2):  # dim-half 1 of q, qc0 window
            fillers.append(lambda q4=q4: proj_qk_group(
                xq_t, wq_sb, bqf_sb, qhT, 1, q4))
        for t in range(8, KT):  # v tiles 8-15: needed at (qc0,hp0) kt 8-15
            fillers.append(lambda t=t: proj_v_group(t))
        for q4 in range(2, 4):  # qc1 window of q, both dim halves
            fillers.append(lambda q4=q4: proj_qk_group(
                xq_t, wq_sb, bqf_sb, qhT, 0, q4))
            fillers.append(lambda q4=q4: proj_qk_group(
                xq_t, wq_sb, bqf_sb, qhT, 1, q4))

        for qc in range(NQH):
            q0 = qc * QW
            for hp in range(2):
                hA, hB = 2 * hp, 2 * hp + 1
                avA = avps.tile([65, QW], F32, name="avA", tag="avA")
                avB = avps.tile([65, QW], F32, name="avB", tag="avB")
                EA, EB = [], []
                for kt in range(KT):
                    psA = scps.tile([128, QW], F32, name="psA", tag="ps")
                    psB = scps.tile([128, QW], F32, name="psB", tag="ps")
                    kcols = slice(kt * 128, (kt + 1) * 128)
                    for nx in (0, 512):
                        nc.tensor.matmul(psA[:, nx:nx + 512],
                                         khT[0:64, hp, kcols],
                                         qhT[0:64, hp, q0 + nx:q0 + nx + 512],
                                         start=True, stop=True)
                    for nx in (0, 512):
                        nc.tensor.matmul(psB[:, nx:nx + 512],
                                         khT[64:128, hp, kcols],
                                         qhT[64:128, hp, q0 + nx:q0 + nx + 512],
                                         start=True, stop=True)
                    eA = epool.tile([128, QW], BF16, name="eA", tag="E")
                    eB = epool.tile([128, QW], BF16, name="eB", tag="E")
                    nc.scalar.activation(eA[:], psA[:], AF.Exp, scale=float(SCALE))
                    nc.scalar.activation(eB[:], psB[:], AF.Exp, scale=float(SCALE))
                    EA.append(eA)
                    EB.append(eB)
                    for nx in (0, 512):
                        nc.tensor.matmul(avA[:, nx:nx + 512], vh[:, kt, hA, :],
                                         eA[:, nx:nx + 512],
                                         start=(kt == 0), stop=(kt == KT - 1))
                    for nx in (0, 512):
                        nc.tensor.matmul(avB[:, nx:nx + 512], vh[:, kt, hB, :],
                                         eB[:, nx:nx + 512],
                                         start=(kt == 0), stop=(kt == KT - 1))
                    pop_fillers(2)
                # evict attention accumulators (frees the PSUM banks) and
                # kick off the reciprocal chain; actual normalization is
                # emitted after PE filler work so the chain never stalls PE.
                evA = evp.tile([65, QW], BF16, name="evA", tag="ev")
                evB = evp.tile([65, QW], BF16, name="evB", tag="ev")
                nc.vector.tensor_copy(evA[:], avA[:])
                nc.vector.tensor_copy(evB[:], avB[:])
                rsp = smol.tile([128, 16], BF16, name="rsp", tag="rsp")
                nc.sync.dma_start(rsp[0:64, :], evA[64:65, :])
                nc.sync.dma_start(rsp[64:128, :], evB[64:65, :])
                rcp = smol.tile([128, 16], F32, name="rcp", tag="rcp")
                nc.vector.reciprocal(rcp[:], rsp[:])
                rcb = smol.tile([128, 16], BF16, name="rcb", tag="rcb")
                nc.vector.tensor_copy(rcb[:], rcp[:])
                rA = smol.tile([1, QW], BF16, name="rA", tag="rA")
                rB = smol.tile([1, QW], BF16, name="rB", tag="rB")
                nc.sync.dma_start(rA[:], rcb[0:64, :])
                nc.sync.dma_start(rB[:], rcb[64:128, :])

                pop_fillers(2)

                # broadcast 1/rowsum to 128 partitions via K=1 matmuls
                rbAp = scps.tile([128, QW], F32, name="rbAp", tag="ps")
                rbBp = scps.tile([128, QW], F32, name="rbBp", tag="ps")
                for nx in (0, 512):
                    nc.tensor.matmul(rbAp[:, nx:nx + 512], ones_bf[0:1, 0:128],
                                     rA[0:1, nx:nx + 512], start=True, stop=True)
                for nx in (0, 512):
                    nc.tensor.matmul(rbBp[:, nx:nx + 512], ones_bf[0:1, 0:128],
                                     rB[0:1, nx:nx + 512], start=True, stop=True)
                rbA = rbp.tile([128, QW], BF16, name="rbA", tag="rb")
                rbB = rbp.tile([128, QW], BF16, name="rbB", tag="rb")
                nc.vector.tensor_copy(rbA[:], rbAp[:])
                nc.vector.tensor_copy(rbB[:], rbBp[:])
                # normalized attention outputs (A_norm^T)
                nc.vector.tensor_mul(an[0:64, hp, q0:q0 + QW],
                                     evA[0:64, :], rbA[0:64, :])
                tmpB = twp.tile([64, QW], BF16, name="tmpB", tag="tw")
                nc.vector.tensor_mul(tmpB[:], evB[0:64, :], rbB[0:64, :])
                nc.gpsimd.dma_start(an[64:128, hp, q0:q0 + QW], tmpB[:])
                # tw partials: normalized per-head prob maps, streamed out
                for kt in range(KT):
                    twA = twp.tile([128, QW], BF16, name="twA", tag="tw")
                    twB = twp.tile([128, QW], BF16, name="twB", tag="tw")
                    nc.vector.tensor_mul(twA[:], EA[kt][:], rbA[:])
                    nc.vector.tensor_mul(twB[:], EB[kt][:], rbB[:])
                    nc.gpsimd.dma_start(
                        tw_d[hA][kt * 128:(kt + 1) * 128, q0:q0 + QW], twA[:])
                    nc.gpsimd.dma_start(
                        tw_d[hB][kt * 128:(kt + 1) * 128, q0:q0 + QW], twB[:])
            # queue this q-window's output projection as PE filler for the
            # next window's loops (emit directly at the very end).
            for m in range(8):
                for qn in range(2):
                    fillers.append(
                        lambda qc=qc, m=m, qn=qn: outproj_group(qc, m, qn))
        pop_fillers(len(fillers))

    nc.compile()
    return nc


def _get_program():
    if "nc" not in _CACHE:
        _CACHE["nc"] = build_program()
    return _CACHE["nc"]


def _bf(x):
    return np.ascontiguousarray(np.asarray(x, dtype=np.float32)).astype(NPBF)


def make_in_maps(q, k, v, Wq, bq, Wk, bk, Wv, bv, Wo):
    in_maps = []
    for c in range(8):
        b, hg = c // 4, c % 4
        sl = slice(hg * EC, (hg + 1) * EC)
        in_maps.append({
            "xqT": _bf(q[b].T), "xkT": _bf(k[b].T), "xvT": _bf(v[b].T),
            "wq": _bf(Wq[:, sl]), "wk": _bf(Wk[:, sl]), "wv": _bf(Wv[:, sl]),
            "wo": _bf(Wo[sl, :]),
            "bqf": np.ascontiguousarray(
                np.asarray(bq[sl], np.float32).reshape(2, 128).T),
            "bkf": np.ascontiguousarray(
                np.asarray(bk[sl], np.float32).reshape(2, 128).T),
            "bv": _bf(bv[sl].reshape(1, EC)),
        })
    return in_maps


def gather_results(results, bo):
    out = np.zeros((2, N, EMB), np.float32)
    tw = np.zeros((N, N), np.float32)
    for c in range(8):
        b = c // 4
        out[b] += results[c]["outT"].astype(np.float32).T
        if b == 0:
            for h in range(HL):
                tw += results[c][f"tw{h}"].astype(np.float32)
    out += np.asarray(bo, dtype=np.float32).reshape(1, 1, EMB)
    return out, np.ascontiguousarray(tw.T)


def kernel(q, k, v, Wq, bq, Wk, bk, Wv, bv, Wo, bo, **run_kwargs):
    nc = _get_program()
    in_maps = make_in_maps(q, k, v, Wq, bq, Wk, bk, Wv, bv, Wo)
    res = run_bass_kernel_spmd(nc, in_maps, list(range(8)), **run_kwargs)
    out, tw = gather_results(res.results, bo)
    if run_kwargs:
        return (out, tw), res
    return (out, tw)


# revision 22
# speedup vs baseline: 1.0704x; 1.0704x over previous
"""Multi-head attention (B=2, N=2048, EMB=1024, 16 heads) on 8 TRN2 NeuronCores.

Sharding: data-parallel over batch (2) x tensor-parallel over heads (4 groups of
4 heads).  Core c handles batch c//4 and heads [4*(c%4), 4*(c%4)+4).  Each core:
  - projects its batch's q/k/v against the column slice of Wq/Wk/Wv for its
    heads (activations pre-transposed on host to [EMB, N] so features sit on
    SBUF partitions),
  - runs attention for its 4 heads in S^T orientation ([k_tokens, q_tokens]):
    softmax row-sums come for free from a ones-column appended to V in the
    P@V matmul; exp runs on the scalar engine straight out of PSUM,
  - applies the row-parallel slice of Wo, producing a partial [EMB, N] output,
  - emits each head's normalized prob map (transposed, bf16) for the
    total_attention_weights diagnostic.
Host sums partial outputs across the 4 cores of each batch (+ bo) and sums the
16 per-head tw partials of batch-0 cores.  Device compute is bf16 with fp32
PSUM accumulation.

Program structure is ordered for engine overlap: the scalar engine's exp
stream is the steady-state bottleneck, so projection work for the second
head-pair and the output-projection matmuls of the previous q-chunk are
interleaved into the attention loops to fill the PE's slack, and the softmax
reciprocal chain (PSUM row -> SBUF -> packed DVE reciprocal -> broadcast
matmul) is deferred behind PE filler work so it never stalls the PE.
"""
import sys

sys.path.insert(0, "/opt/trn_rl_repo")

from contextlib import ExitStack

import numpy as np
import ml_dtypes

import concourse.bass as bass
import concourse.bacc as bacc
import concourse.tile as tile
from concourse import mybir
from concourse.bass_utils import run_bass_kernel_spmd

BF16 = mybir.dt.bfloat16
F32 = mybir.dt.float32
AF = mybir.ActivationFunctionType
NPBF = ml_dtypes.bfloat16

N = 2048          # sequence length
EMB = 1024        # model dim
HL = 4            # heads per core
D = 64            # head dim
EC = HL * D       # head dims per core (256)
KT = N // 128     # 16 k-token tiles
QW = 1024         # q chunk width
NQH = N // QW     # 2 q chunks
SCALE = 1.0 / np.sqrt(D)

_CACHE = {}


def build_program():
    nc = bacc.Bacc("TRN2", target_bir_lowering=False, debug=False, num_devices=8)

    xq_d = nc.declare_dram_parameter("xqT", [EMB, N], BF16, isOutput=False)
    xk_d = nc.declare_dram_parameter("xkT", [EMB, N], BF16, isOutput=False)
    xv_d = nc.declare_dram_parameter("xvT", [EMB, N], BF16, isOutput=False)
    wq_d = nc.declare_dram_parameter("wq", [EMB, EC], BF16, isOutput=False)
    wk_d = nc.declare_dram_parameter("wk", [EMB, EC], BF16, isOutput=False)
    wv_d = nc.declare_dram_parameter("wv", [EMB, EC], BF16, isOutput=False)
    wo_d = nc.declare_dram_parameter("wo", [EC, EMB], BF16, isOutput=False)
    bqf_d = nc.declare_dram_parameter("bqf", [128, 2], F32, isOutput=False)
    bkf_d = nc.declare_dram_parameter("bkf", [128, 2], F32, isOutput=False)
    bv_d = nc.declare_dram_parameter("bv", [1, EC], BF16, isOutput=False)
    out_d = nc.declare_dram_parameter("outT", [EMB, N], BF16, isOutput=True)
    tw_d = [nc.declare_dram_parameter(f"tw{h}", [N, N], BF16, isOutput=True)
            for h in range(HL)]

    with ExitStack() as ctx:
        tc = ctx.enter_context(tile.TileContext(nc))

        const = ctx.enter_context(tc.tile_pool(name="const", bufs=1))
        ones_bf = const.tile([1, 512], BF16, name="ones_bf")
        nc.vector.memset(ones_bf[:], 1.0)
        bqf_sb = const.tile([128, 2], F32, name="bqf_sb")
        bkf_sb = const.tile([128, 2], F32, name="bkf_sb")
        bv_sb = const.tile([1, EC], BF16, name="bv_sb")
        nc.sync.dma_start(bqf_sb[:], bqf_d[:])
        nc.sync.dma_start(bkf_sb[:], bkf_d[:])
        nc.sync.dma_start(bv_sb[:], bv_d[:])

        wpool = ctx.enter_context(tc.tile_pool(name="wpool", bufs=1))
        wq_sb = wpool.tile([128, 8, EC], BF16, name="wq_sb")  # k-chunk major
        wk_sb = wpool.tile([128, 8, EC], BF16, name="wk_sb")
        wv_sb = wpool.tile([128, 8, EC], BF16, name="wv_sb")
        wo_sb = wpool.tile([128, 2, EMB], BF16, name="wo_sb")
        for kc in range(8):
            nc.sync.dma_start(wk_sb[:, kc, :], wk_d[kc * 128:(kc + 1) * 128, :])
            nc.sync.dma_start(wq_sb[:, kc, :], wq_d[kc * 128:(kc + 1) * 128, :])
            nc.sync.dma_start(wv_sb[:, kc, :], wv_d[kc * 128:(kc + 1) * 128, :])
        for hc in range(2):
            nc.sync.dma_start(wo_sb[:, hc, :], wo_d[hc * 128:(hc + 1) * 128, :])

        qkv = ctx.enter_context(tc.tile_pool(name="qkv", bufs=1))
        qhT = qkv.tile([128, 2, N], BF16, name="qhT")      # [dim-half][128, tok]
        khT = qkv.tile([128, 2, N], BF16, name="khT")
        vh = qkv.tile([128, KT, HL, 65], BF16, name="vh")  # [tok128][kt][h][64+ones]
        an = qkv.tile([128, 2, N], BF16, name="an")        # A_norm^T [hd-chunk][q]
        nc.vector.memset(vh[:, :, :, 64:65], 1.0)

        # 16 slots: xk tiles are fully consumed by the upfront k-projections,
        # so the xv tiles recycle their slots; xq tiles stay live for the
        # deferred q-projection fillers.
        xpool = ctx.enter_context(tc.tile_pool(name="xpool", bufs=16))
        # Single shared PSUM pool for projection / scores / broadcast /
        # out-projection groups (tag "ps", 2 slots x 2 banks) + the attention
        # accumulators (4 banks) = all 8 PSUM banks.
        scps = ctx.enter_context(tc.tile_pool(name="scps", bufs=2, space="PSUM"))
        avps = ctx.enter_context(tc.tile_pool(name="avps", bufs=1, space="PSUM"))

        # ---- phase A: DMAs + projections (emitted piecewise for overlap) ----
        def load_x(x_d, pfx):
            xt = []
            for kc in range(8):
                xtile = xpool.tile([128, N], BF16, name=f"{pfx}{kc}", tag="x")
                nc.sync.dma_start(xtile[:], x_d[kc * 128:(kc + 1) * 128, :])
                xt.append(xtile)
            return xt

        xk_t = load_x(xk_d, "xk")
        xq_t = load_x(xq_d, "xq")
        xv_t = load_x(xv_d, "xv")

        def proj_qk_group(xt, w_sb, bf_sb, dst, dh, q4):
            ps = scps.tile([128, 512], F32, name="pj", tag="ps")
            for kc in range(8):
                nc.tensor.matmul(
                    ps[:], w_sb[:, kc, dh * 128:(dh + 1) * 128],
                    xt[kc][:, q4 * 512:(q4 + 1) * 512],
                    start=(kc == 0), stop=(kc == 7))
            nc.scalar.activation(dst[:, dh, q4 * 512:(q4 + 1) * 512], ps[:],
                                 AF.Identity, bias=bf_sb[:, dh:dh + 1], scale=1.0)

        def proj_v_group(t):
            ps = scps.tile([128, EC], F32, name="pv", tag="ps")
            for kc in range(8):
                nc.tensor.matmul(ps[:], xv_t[kc][:, t * 128:(t + 1) * 128],
                                 wv_sb[:, kc, :], start=(kc == 0), stop=False)
            nc.tensor.matmul(ps[:], ones_bf[0:1, 0:128], bv_sb[0:1, :],
                             start=False, stop=True)
            nc.scalar.copy(vh[:, t, :, 0:64],
                           ps[:].rearrange("p (h c) -> p h c", h=HL))

        # upfront: all of k (so the xk tiles can be recycled for xv), the qc0
        # window of q dim-half 0, and the first v tiles.
        for q4 in range(4):
            proj_qk_group(xk_t, wk_sb, bkf_sb, khT, 0, q4)
        for q4 in range(4):
            proj_qk_group(xk_t, wk_sb, bkf_sb, khT, 1, q4)
        for q4 in range(2):
            proj_qk_group(xq_t, wq_sb, bqf_sb, qhT, 0, q4)
        for t in range(8):
            proj_v_group(t)
        epool = ctx.enter_context(tc.tile_pool(name="epool", bufs=33))
        evp = ctx.enter_context(tc.tile_pool(name="evp", bufs=3))
        rbp = ctx.enter_context(tc.tile_pool(name="rbp", bufs=3))
        twp = ctx.enter_context(tc.tile_pool(name="twp", bufs=3))
        smol = ctx.enter_context(tc.tile_pool(name="smol", bufs=2))

        def outproj_group(qc, m, qn):
            """One output-projection PSUM group for q window qc (512 cols)."""
            q0 = qc * QW + qn * 512
            ps = scps.tile([128, 512], F32, name="op", tag="ps")
            for hc in range(2):
                nc.tensor.matmul(ps[:], wo_sb[:, hc, m * 128:(m + 1) * 128],
                                 an[:, hc, q0:q0 + 512],
                                 start=(hc == 0), stop=(hc == 1))
            ot = twp.tile([128, 512], BF16, name="ot", tag="tw")
            if (m + qn) % 2:
                nc.scalar.copy(ot[:], ps[:])
            else:
                nc.vector.tensor_copy(ot[:], ps[:])
            nc.gpsimd.dma_start(
                out_d[m * 128:(m + 1) * 128, q0:q0 + 512], ot[:])

        # PE filler queue: closures emitting matmul-heavy groups, injected
        # into attention-loop slack so the PE never idles cold.  FIFO order
        # respects each deferred group's deadline (first use downstream).
        fillers = []

        def pop_fillers(k):
            for _ in range(min(k, len(fillers))):
                fillers.pop(0)()

        # tw-multiply stream of the previous head-pair, paced one k-tile per
        # loop iteration so it never monopolizes the DVE FIFO (which would
        # hold PSUM slots hostage and starve PE/ACT).
        pending_tw = []

        def pop_tw(k):
            for _ in range(min(k, len(pending_tw))):
                pending_tw.pop(0)()

        for q4 in range(2):  # dim-half 1 of q, qc0 window
            fillers.append(lambda q4=q4: proj_qk_group(
                xq_t, wq_sb, bqf_sb, qhT, 1, q4))
        for t in range(8, KT):  # v tiles 8-15: needed at (qc0,hp0) kt 8-15
            fillers.append(lambda t=t: proj_v_group(t))
        for q4 in range(2, 4):  # qc1 window of q, both dim halves
            fillers.append(lambda q4=q4: proj_qk_group(
                xq_t, wq_sb, bqf_sb, qhT, 0, q4))
            fillers.append(lambda q4=q4: proj_qk_group(
                xq_t, wq_sb, bqf_sb, qhT, 1, q4))

        for qc in range(NQH):
            q0 = qc * QW
            for hp in range(2):
                hA, hB = 2 * hp, 2 * hp + 1
                avA = avps.tile([65, QW], F32, name="avA", tag="avA")
                avB = avps.tile([65, QW], F32, name="avB", tag="avB")
                EA, EB = [], []
                for kt in range(KT):
                    psA = scps.tile([128, QW], F32, name="psA", tag="ps")
                    psB = scps.tile([128, QW], F32, name="psB", tag="ps")
                    kcols = slice(kt * 128, (kt + 1) * 128)
                    for nx in (0, 512):
                        nc.tensor.matmul(psA[:, nx:nx + 512],
                                         khT[0:64, hp, kcols],
                                         qhT[0:64, hp, q0 + nx:q0 + nx + 512],
                                         start=True, stop=True)
                    for nx in (0, 512):
                        nc.tensor.matmul(psB[:, nx:nx + 512],
                                         khT[64:128, hp, kcols],
                                         qhT[64:128, hp, q0 + nx:q0 + nx + 512],
                                         start=True, stop=True)
                    pop_tw(1)  # frees the E slots the next exps will claim
                    eA = epool.tile([128, QW], BF16, name="eA", tag="E")
                    eB = epool.tile([128, QW], BF16, name="eB", tag="E")
                    nc.scalar.activation(eA[:], psA[:], AF.Exp, scale=float(SCALE))
                    nc.scalar.activation(eB[:], psB[:], AF.Exp, scale=float(SCALE))
                    EA.append(eA)
                    EB.append(eB)
                    for nx in (0, 512):
                        nc.tensor.matmul(avA[:, nx:nx + 512], vh[:, kt, hA, :],
                                         eA[:, nx:nx + 512],
                                         start=(kt == 0), stop=(kt == KT - 1))
                    for nx in (0, 512):
                        nc.tensor.matmul(avB[:, nx:nx + 512], vh[:, kt, hB, :],
                                         eB[:, nx:nx + 512],
                                         start=(kt == 0), stop=(kt == KT - 1))
                    if kt < 12:
                        pop_fillers(1)
                # evict attention accumulators (freeing their PSUM banks and
                # exposing the rowsum rows to DMA) and run the reciprocal
                # chain; PE filler work covers the chain's latency.
                evA = evp.tile([65, QW], BF16, name="evA", tag="ev")
                evB = evp.tile([65, QW], BF16, name="evB", tag="ev")
                nc.vector.tensor_copy(evA[:], avA[:])
                nc.scalar.copy(evB[:], avB[:])
                rsp = smol.tile([128, 16], BF16, name="rsp", tag="rsp")
                nc.sync.dma_start(rsp[0:64, :], evA[64:65, :])
                nc.sync.dma_start(rsp[64:128, :], evB[64:65, :])
                rcp = smol.tile([128, 16], F32, name="rcp", tag="rcp")
                nc.vector.reciprocal(rcp[:], rsp[:])
                rcb = smol.tile([128, 16], BF16, name="rcb", tag="rcb")
                nc.vector.tensor_copy(rcb[:], rcp[:])
                rA = smol.tile([1, QW], BF16, name="rA", tag="rA")
                rB = smol.tile([1, QW], BF16, name="rB", tag="rB")
                nc.sync.dma_start(rA[:], rcb[0:64, :])
                nc.sync.dma_start(rB[:], rcb[64:128, :])

                pop_tw(len(pending_tw))
                pop_fillers(2)

                # broadcast 1/rowsum to 128 partitions via K=1 matmuls
                rbAp = scps.tile([128, QW], F32, name="rbAp", tag="ps")
                rbBp = scps.tile([128, QW], F32, name="rbBp", tag="ps")
                for nx in (0, 512):
                    nc.tensor.matmul(rbAp[:, nx:nx + 512], ones_bf[0:1, 0:128],
                                     rA[0:1, nx:nx + 512], start=True, stop=True)
                for nx in (0, 512):
                    nc.tensor.matmul(rbBp[:, nx:nx + 512], ones_bf[0:1, 0:128],
                                     rB[0:1, nx:nx + 512], start=True, stop=True)
                rbA = rbp.tile([128, QW], BF16, name="rbA", tag="rb")
                rbB = rbp.tile([128, QW], BF16, name="rbB", tag="rb")
                nc.vector.tensor_copy(rbA[:], rbAp[:])
                nc.vector.tensor_copy(rbB[:], rbBp[:])
                # normalized attention outputs (A_norm^T)
                nc.vector.tensor_mul(an[0:64, hp, q0:q0 + QW],
                                     evA[0:64, :], rbA[0:64, :])
                tmpB = twp.tile([64, QW], BF16, name="tmpB", tag="tw")
                nc.vector.tensor_mul(tmpB[:], evB[0:64, :], rbB[0:64, :])
                nc.gpsimd.dma_start(an[64:128, hp, q0:q0 + QW], tmpB[:])

                # tw partials (normalized per-head prob maps): deferred,
                # drained one k-tile per iteration of the next loop.
                def tw_step(kt, hA=hA, hB=hB, q0=q0, EA=EA, EB=EB,
                            rbA=rbA, rbB=rbB):
                    twA = twp.tile([128, QW], BF16, name="twA", tag="tw")
                    twB = twp.tile([128, QW], BF16, name="twB", tag="tw")
                    nc.vector.tensor_mul(twA[:], EA[kt][:], rbA[:])
                    nc.vector.tensor_mul(twB[:], EB[kt][:], rbB[:])
                    nc.gpsimd.dma_start(
                        tw_d[hA][kt * 128:(kt + 1) * 128, q0:q0 + QW], twA[:])
                    nc.gpsimd.dma_start(
                        tw_d[hB][kt * 128:(kt + 1) * 128, q0:q0 + QW], twB[:])

                assert not pending_tw
                for kt in range(KT):
                    pending_tw.append(lambda kt=kt: tw_step(kt))
            # queue this q-window's output projection as PE filler for the
            # next window's loops.
            for m in range(8):
                for qn in range(2):
                    fillers.append(
                        lambda qc=qc, m=m, qn=qn: outproj_group(qc, m, qn))
        while pending_tw or fillers:
            pop_tw(1)
            pop_fillers(1)

    nc.compile()
    return nc


def _get_program():
    if "nc" not in _CACHE:
        _CACHE["nc"] = build_program()
    return _CACHE["nc"]


def _bf(x):
    return np.ascontiguousarray(np.asarray(x, dtype=np.float32)).astype(NPBF)


def make_in_maps(q, k, v, Wq, bq, Wk, bk, Wv, bv, Wo):
    in_maps = []
    for c in range(8):
        b, hg = c // 4, c % 4
        sl = slice(hg * EC, (hg + 1) * EC)
        in_maps.append({
            "xqT": _bf(q[b].T), "xkT": _bf(k[b].T), "xvT": _bf(v[b].T),
            "wq": _bf(Wq[:, sl]), "wk": _bf(Wk[:, sl]), "wv": _bf(Wv[:, sl]),
            "wo": _bf(Wo[sl, :]),
            "bqf": np.ascontiguousarray(
                np.asarray(bq[sl], np.float32).reshape(2, 128).T),
            "bkf": np.ascontiguousarray(
                np.asarray(bk[sl], np.float32).reshape(2, 128).T),
            "bv": _bf(bv[sl].reshape(1, EC)),
        })
    return in_maps


def gather_results(results, bo):
    out = np.zeros((2, N, EMB), np.float32)
    tw = np.zeros((N, N), np.float32)
    for c in range(8):
        b = c // 4
        out[b] += results[c]["outT"].astype(np.float32).T
        if b == 0:
            for h in range(HL):
                tw += results[c][f"tw{h}"].astype(np.float32)
    out += np.asarray(bo, dtype=np.float32).reshape(1, 1, EMB)
    return out, np.ascontiguousarray(tw.T)


def kernel(q, k, v, Wq, bq, Wk, bk, Wv, bv, Wo, bo, **run_kwargs):
    nc = _get_program()
    in_maps = make_in_maps(q, k, v, Wq, bq, Wk, bk, Wv, bv, Wo)
    res = run_bass_kernel_spmd(nc, in_maps, list(range(8)), **run_kwargs)
    out, tw = gather_results(res.results, bo)
    if run_kwargs:
        return (out, tw), res
    return (out, tw)


# revision 23
# speedup vs baseline: 1.0913x; 1.0195x over previous
"""Multi-head attention (B=2, N=2048, EMB=1024, 16 heads) on 8 TRN2 NeuronCores.

Sharding: data-parallel over batch (2) x tensor-parallel over heads (4 groups of
4 heads).  Core c handles batch c//4 and heads [4*(c%4), 4*(c%4)+4).  Each core:
  - projects its batch's q/k/v against the column slice of Wq/Wk/Wv for its
    heads (activations pre-transposed on host to [EMB, N] so features sit on
    SBUF partitions),
  - runs attention for its 4 heads in S^T orientation ([k_tokens, q_tokens]):
    softmax row-sums come for free from a ones-column appended to V in the
    P@V matmul; exp runs on the scalar engine straight out of PSUM,
  - applies the row-parallel slice of Wo, producing a partial [EMB, N] output,
  - emits each head's normalized prob map (transposed, bf16) for the
    total_attention_weights diagnostic.
Host sums partial outputs across the 4 cores of each batch (+ bo) and sums the
16 per-head tw partials of batch-0 cores.  Device compute is bf16 with fp32
PSUM accumulation.

Scheduling notes: the scalar engine's exp stream (2 x [128,1024] per k-tile)
is the steady-state bottleneck, so everything else hides under it.  Each
head-pair's normalization block (reciprocal-broadcast matmuls + the DVE
multiply stream producing A_norm and the tw maps) is deferred three k-tiles
into the NEXT head-pair's loop so its DMA-reciprocal latency never stalls the
PE FIFO; the attention-accumulator eviction happens immediately (on DVE and
ACT) so the PSUM banks and rowsum rows are available right away.
"""
import sys

sys.path.insert(0, "/opt/trn_rl_repo")

from contextlib import ExitStack

import numpy as np
import ml_dtypes

import concourse.bass as bass
import concourse.bacc as bacc
import concourse.tile as tile
from concourse import mybir
from concourse.bass_utils import run_bass_kernel_spmd

BF16 = mybir.dt.bfloat16
F32 = mybir.dt.float32
AF = mybir.ActivationFunctionType
NPBF = ml_dtypes.bfloat16

N = 2048          # sequence length
EMB = 1024        # model dim
HL = 4            # heads per core
D = 64            # head dim
EC = HL * D       # head dims per core (256)
KT = N // 128     # 16 k-token tiles
QW = 1024         # q chunk width
NQH = N // QW     # 2 q chunks
SCALE = 1.0 / np.sqrt(D)

_CACHE = {}


def build_program():
    nc = bacc.Bacc("TRN2", target_bir_lowering=False, debug=False, num_devices=8)

    xq_d = nc.declare_dram_parameter("xqT", [EMB, N], BF16, isOutput=False)
    xk_d = nc.declare_dram_parameter("xkT", [EMB, N], BF16, isOutput=False)
    xv_d = nc.declare_dram_parameter("xvT", [EMB, N], BF16, isOutput=False)
    wq_d = nc.declare_dram_parameter("wq", [EMB, EC], BF16, isOutput=False)
    wk_d = nc.declare_dram_parameter("wk", [EMB, EC], BF16, isOutput=False)
    wv_d = nc.declare_dram_parameter("wv", [EMB, EC], BF16, isOutput=False)
    wo_d = nc.declare_dram_parameter("wo", [EC, EMB], BF16, isOutput=False)
    bqf_d = nc.declare_dram_parameter("bqf", [128, 2], F32, isOutput=False)
    bkf_d = nc.declare_dram_parameter("bkf", [128, 2], F32, isOutput=False)
    bv_d = nc.declare_dram_parameter("bv", [1, EC], BF16, isOutput=False)
    out_d = nc.declare_dram_parameter("outT", [EMB, N], BF16, isOutput=True)
    tw_d = [nc.declare_dram_parameter(f"tw{h}", [N, N], BF16, isOutput=True)
            for h in range(HL)]

    with ExitStack() as ctx:
        tc = ctx.enter_context(tile.TileContext(nc))

        const = ctx.enter_context(tc.tile_pool(name="const", bufs=1))
        ones_bf = const.tile([1, 512], BF16, name="ones_bf")
        nc.vector.memset(ones_bf[:], 1.0)
        bqf_sb = const.tile([128, 2], F32, name="bqf_sb")
        bkf_sb = const.tile([128, 2], F32, name="bkf_sb")
        bv_sb = const.tile([1, EC], BF16, name="bv_sb")
        nc.sync.dma_start(bqf_sb[:], bqf_d[:])
        nc.sync.dma_start(bkf_sb[:], bkf_d[:])
        nc.sync.dma_start(bv_sb[:], bv_d[:])

        wpool = ctx.enter_context(tc.tile_pool(name="wpool", bufs=1))
        wq_sb = wpool.tile([128, 8, EC], BF16, name="wq_sb")  # k-chunk major
        wk_sb = wpool.tile([128, 8, EC], BF16, name="wk_sb")
        wv_sb = wpool.tile([128, 8, EC], BF16, name="wv_sb")
        wo_sb = wpool.tile([128, 2, EMB], BF16, name="wo_sb")
        for kc in range(8):
            nc.sync.dma_start(wk_sb[:, kc, :], wk_d[kc * 128:(kc + 1) * 128, :])
            nc.sync.dma_start(wq_sb[:, kc, :], wq_d[kc * 128:(kc + 1) * 128, :])
            nc.sync.dma_start(wv_sb[:, kc, :], wv_d[kc * 128:(kc + 1) * 128, :])
        for hc in range(2):
            nc.sync.dma_start(wo_sb[:, hc, :], wo_d[hc * 128:(hc + 1) * 128, :])

        qkv = ctx.enter_context(tc.tile_pool(name="qkv", bufs=1))
        qhT = qkv.tile([128, 2, N], BF16, name="qhT")      # [dim-half][128, tok]
        khT = qkv.tile([128, 2, N], BF16, name="khT")
        vh = qkv.tile([128, KT, HL, 65], BF16, name="vh")  # [tok128][kt][h][64+ones]
        an = qkv.tile([128, 2, N], BF16, name="an")        # A_norm^T [hd-chunk][q]
        nc.vector.memset(vh[:, :, :, 64:65], 1.0)

        # ---------------- phase A: projections ----------------
        with tc.tile_pool(name="xpool", bufs=24) as xpool, \
             tc.tile_pool(name="prps", bufs=4, space="PSUM") as prps:

            def load_x(x_d, pfx):
                xt = []
                for kc in range(8):
                    xtile = xpool.tile([128, N], BF16, name=f"{pfx}{kc}", tag="x")
                    nc.sync.dma_start(xtile[:], x_d[kc * 128:(kc + 1) * 128, :])
                    xt.append(xtile)
                return xt

            xk_t = load_x(xk_d, "xk")
            xq_t = load_x(xq_d, "xq")
            xv_t = load_x(xv_d, "xv")

            def proj_qk_group(xt, w_sb, bf_sb, dst, dh, q4):
                ps = prps.tile([128, 512], F32, name="pj", tag="prps")
                for kc in range(8):
                    nc.tensor.matmul(
                        ps[:], w_sb[:, kc, dh * 128:(dh + 1) * 128],
                        xt[kc][:, q4 * 512:(q4 + 1) * 512],
                        start=(kc == 0), stop=(kc == 7))
                nc.scalar.activation(dst[:, dh, q4 * 512:(q4 + 1) * 512], ps[:],
                                     AF.Identity, bias=bf_sb[:, dh:dh + 1],
                                     scale=1.0)

            def proj_v_group(t):
                ps = prps.tile([128, EC], F32, name="pv", tag="prps")
                for kc in range(8):
                    nc.tensor.matmul(ps[:], xv_t[kc][:, t * 128:(t + 1) * 128],
                                     wv_sb[:, kc, :], start=(kc == 0), stop=False)
                nc.tensor.matmul(ps[:], ones_bf[0:1, 0:128], bv_sb[0:1, :],
                                 start=False, stop=True)
                nc.scalar.copy(vh[:, t, :, 0:64],
                               ps[:].rearrange("p (h c) -> p h c", h=HL))

            for q4 in range(4):
                proj_qk_group(xk_t, wk_sb, bkf_sb, khT, 0, q4)
            for q4 in range(2):
                proj_qk_group(xq_t, wq_sb, bqf_sb, qhT, 0, q4)
            for t in range(KT):
                proj_v_group(t)
            for q4 in range(4):
                proj_qk_group(xk_t, wk_sb, bkf_sb, khT, 1, q4)
            for q4 in range(2, 4):
                proj_qk_group(xq_t, wq_sb, bqf_sb, qhT, 0, q4)
            for q4 in range(4):
                proj_qk_group(xq_t, wq_sb, bqf_sb, qhT, 1, q4)

        # ---------------- phase B: attention ----------------
        scps = ctx.enter_context(tc.tile_pool(name="scps", bufs=2, space="PSUM"))
        avps = ctx.enter_context(tc.tile_pool(name="avps", bufs=1, space="PSUM"))
        epool = ctx.enter_context(tc.tile_pool(name="epool", bufs=38))
        evp = ctx.enter_context(tc.tile_pool(name="evp", bufs=3))
        rbp = ctx.enter_context(tc.tile_pool(name="rbp", bufs=3))
        twp = ctx.enter_context(tc.tile_pool(name="twp", bufs=4))
        otp = ctx.enter_context(tc.tile_pool(name="otp", bufs=3))
        smol = ctx.enter_context(tc.tile_pool(name="smol", bufs=2))

        def outproj_group(qc, m, qn):
            """Output projection for one [128, 512] tile of out^T."""
            q0 = qc * QW + qn * 512
            ps = scps.tile([128, 512], F32, name="op", tag="ps")
            for hc in range(2):
                nc.tensor.matmul(ps[:], wo_sb[:, hc, m * 128:(m + 1) * 128],
                                 an[:, hc, q0:q0 + 512],
                                 start=(hc == 0), stop=(hc == 1))
            ot = otp.tile([128, 512], BF16, name="ot", tag="ot")
            if (m + qn) % 2:
                nc.scalar.copy(ot[:], ps[:])
            else:
                nc.vector.tensor_copy(ot[:], ps[:])
            nc.gpsimd.dma_start(
                out_d[m * 128:(m + 1) * 128, q0:q0 + 512], ot[:])

        def make_norm_block(hp, q0, evA, evB, rA, rB, EA, EB):
            hA, hB = 2 * hp, 2 * hp + 1

            def norm_block():
                # broadcast 1/rowsum to 128 partitions via K=1 matmuls
                rbAp = scps.tile([128, QW], F32, name="rbAp", tag="ps")
                rbBp = scps.tile([128, QW], F32, name="rbBp", tag="ps")
                for nx in (0, 512):
                    nc.tensor.matmul(rbAp[:, nx:nx + 512], ones_bf[0:1, 0:128],
                                     rA[0:1, nx:nx + 512], start=True, stop=True)
                for nx in (0, 512):
                    nc.tensor.matmul(rbBp[:, nx:nx + 512], ones_bf[0:1, 0:128],
                                     rB[0:1, nx:nx + 512], start=True, stop=True)
                rbA = rbp.tile([128, QW], BF16, name="rbA", tag="rb")
                rbB = rbp.tile([128, QW], BF16, name="rbB", tag="rb")
                nc.vector.tensor_copy(rbA[:], rbAp[:])
                nc.vector.tensor_copy(rbB[:], rbBp[:])
                # normalized attention outputs (A_norm^T)
                nc.vector.tensor_mul(an[0:64, hp, q0:q0 + QW],
                                     evA[0:64, :], rbA[0:64, :])
                tmpB = twp.tile([64, QW], BF16, name="tmpB", tag="tw")
                nc.vector.tensor_mul(tmpB[:], evB[0:64, :], rbB[0:64, :])
                nc.gpsimd.dma_start(an[64:128, hp, q0:q0 + QW], tmpB[:])
                # tw partials: normalized per-head prob maps, streamed out
                for kt in range(KT):
                    twA = twp.tile([128, QW], BF16, name="twA", tag="tw")
                    twB = twp.tile([128, QW], BF16, name="twB", tag="tw")
                    nc.vector.tensor_mul(twA[:], EA[kt][:], rbA[:])
                    nc.vector.tensor_mul(twB[:], EB[kt][:], rbB[:])
                    nc.gpsimd.dma_start(
                        tw_d[hA][kt * 128:(kt + 1) * 128, q0:q0 + QW], twA[:])
                    nc.gpsimd.dma_start(
                        tw_d[hB][kt * 128:(kt + 1) * 128, q0:q0 + QW], twB[:])

            return norm_block

        pending_norm = None
        for qc in range(NQH):
            q0 = qc * QW
            for hp in range(2):
                hA, hB = 2 * hp, 2 * hp + 1
                avA = avps.tile([65, QW], F32, name="avA", tag="avA")
                avB = avps.tile([65, QW], F32, name="avB", tag="avB")
                EA, EB = [], []
                for kt in range(KT):
                    psA = scps.tile([128, QW], F32, name="psA", tag="ps")
                    psB = scps.tile([128, QW], F32, name="psB", tag="ps")
                    kcols = slice(kt * 128, (kt + 1) * 128)
                    for nx in (0, 512):
                        nc.tensor.matmul(psA[:, nx:nx + 512],
                                         khT[0:64, hp, kcols],
                                         qhT[0:64, hp, q0 + nx:q0 + nx + 512],
                                         start=True, stop=True)
                    for nx in (0, 512):
                        nc.tensor.matmul(psB[:, nx:nx + 512],
                                         khT[64:128, hp, kcols],
                                         qhT[64:128, hp, q0 + nx:q0 + nx + 512],
                                         start=True, stop=True)
                    eA = epool.tile([128, QW], BF16, name="eA", tag="E")
                    eB = epool.tile([128, QW], BF16, name="eB", tag="E")
                    nc.scalar.activation(eA[:], psA[:], AF.Exp, scale=float(SCALE))
                    nc.scalar.activation(eB[:], psB[:], AF.Exp, scale=float(SCALE))
                    EA.append(eA)
                    EB.append(eB)
                    for nx in (0, 512):
                        nc.tensor.matmul(avA[:, nx:nx + 512], vh[:, kt, hA, :],
                                         eA[:, nx:nx + 512],
                                         start=(kt == 0), stop=(kt == KT - 1))
                    for nx in (0, 512):
                        nc.tensor.matmul(avB[:, nx:nx + 512], vh[:, kt, hB, :],
                                         eB[:, nx:nx + 512],
                                         start=(kt == 0), stop=(kt == KT - 1))
                    if kt == 2 and pending_norm is not None:
                        pending_norm()
                        pending_norm = None
                # boundary: evict the accumulators (freeing their PSUM banks,
                # exposing rowsum rows) and run the packed-reciprocal chain.
                evA = evp.tile([65, QW], BF16, name="evA", tag="ev")
                evB = evp.tile([65, QW], BF16, name="evB", tag="ev")
                nc.vector.tensor_copy(evA[:], avA[:])
                nc.scalar.copy(evB[:], avB[:])
                rsp = smol.tile([128, 16], BF16, name="rsp", tag="rsp")
                nc.sync.dma_start(rsp[0:64, :], evA[64:65, :])
                nc.sync.dma_start(rsp[64:128, :], evB[64:65, :])
                rcp = smol.tile([128, 16], F32, name="rcp", tag="rcp")
                nc.vector.reciprocal(rcp[:], rsp[:])
                rcb = smol.tile([128, 16], BF16, name="rcb", tag="rcb")
                nc.vector.tensor_copy(rcb[:], rcp[:])
                rA = smol.tile([1, QW], BF16, name="rA", tag="rA")
                rB = smol.tile([1, QW], BF16, name="rB", tag="rB")
                nc.sync.dma_start(rA[:], rcb[0:64, :])
                nc.sync.dma_start(rB[:], rcb[64:128, :])
                pending_norm = make_norm_block(hp, q0, evA, evB, rA, rB, EA, EB)

        # ---------------- tail: output projections + last norm ----------------
        for m in range(8):
            for qn in range(2):
                outproj_group(0, m, qn)
        pending_norm()
        for m in range(8):
            for qn in range(2):
                outproj_group(1, m, qn)

    nc.compile()
    return nc


def _get_program():
    if "nc" not in _CACHE:
        _CACHE["nc"] = build_program()
    return _CACHE["nc"]


def _bf(x):
    return np.ascontiguousarray(np.asarray(x, dtype=np.float32)).astype(NPBF)


def make_in_maps(q, k, v, Wq, bq, Wk, bk, Wv, bv, Wo):
    in_maps = []
    for c in range(8):
        b, hg = c // 4, c % 4
        sl = slice(hg * EC, (hg + 1) * EC)
        in_maps.append({
            "xqT": _bf(q[b].T), "xkT": _bf(k[b].T), "xvT": _bf(v[b].T),
            "wq": _bf(Wq[:, sl]), "wk": _bf(Wk[:, sl]), "wv": _bf(Wv[:, sl]),
            "wo": _bf(Wo[sl, :]),
            "bqf": np.ascontiguousarray(
                np.asarray(bq[sl], np.float32).reshape(2, 128).T),
            "bkf": np.ascontiguousarray(
                np.asarray(bk[sl], np.float32).reshape(2, 128).T),
            "bv": _bf(bv[sl].reshape(1, EC)),
        })
    return in_maps


def gather_results(results, bo):
    out = np.zeros((2, N, EMB), np.float32)
    tw = np.zeros((N, N), np.float32)
    for c in range(8):
        b = c // 4
        out[b] += results[c]["outT"].astype(np.float32).T
        if b == 0:
            for h in range(HL):
                tw += results[c][f"tw{h}"].astype(np.float32)
    out += np.asarray(bo, dtype=np.float32).reshape(1, 1, EMB)
    return out, np.ascontiguousarray(tw.T)


def kernel(q, k, v, Wq, bq, Wk, bk, Wv, bv, Wo, bo, **run_kwargs):
    nc = _get_program()
    in_maps = make_in_maps(q, k, v, Wq, bq, Wk, bk, Wv, bv, Wo)
    res = run_bass_kernel_spmd(nc, in_maps, list(range(8)), **run_kwargs)
    out, tw = gather_results(res.results, bo)
    if run_kwargs:
        return (out, tw), res
    return (out, tw)


# revision 25
# speedup vs baseline: 1.1027x; 1.0104x over previous
"""Multi-head attention (B=2, N=2048, EMB=1024, 16 heads) on 8 TRN2 NeuronCores.

Sharding: data-parallel over batch (2) x tensor-parallel over heads (4 groups of
4 heads).  Core c handles batch c//4 and heads [4*(c%4), 4*(c%4)+4).  Each core:
  - projects its batch's q/k/v against the column slice of Wq/Wk/Wv for its
    heads (activations pre-transposed on host to [EMB, N] so features sit on
    SBUF partitions),
  - runs attention for its 4 heads in S^T orientation ([k_tokens, q_tokens]):
    softmax row-sums come for free from a ones-column appended to V in the
    P@V matmul; exp runs on the scalar engine straight out of PSUM,
  - applies the row-parallel slice of Wo, producing a partial [EMB, N] output,
  - emits each head's normalized prob map (transposed, bf16) for the
    total_attention_weights diagnostic.
Host sums partial outputs across the 4 cores of each batch (+ bo) and sums the
16 per-head tw partials of batch-0 cores.  Device compute is bf16 with fp32
PSUM accumulation.

Scheduling notes: the scalar engine's exp stream (2 x [128,1024] per k-tile)
is the steady-state bottleneck, so everything else hides under it.  Each
head-pair's normalization block (reciprocal-broadcast matmuls + the DVE
multiply stream producing A_norm and the tw maps) is deferred three k-tiles
into the NEXT head-pair's loop so its DMA-reciprocal latency never stalls the
PE FIFO; the attention-accumulator eviction happens immediately (on DVE and
ACT) so the PSUM banks and rowsum rows are available right away.
"""
import sys

sys.path.insert(0, "/opt/trn_rl_repo")

from contextlib import ExitStack

import numpy as np
import ml_dtypes

import concourse.bass as bass
import concourse.bacc as bacc
import concourse.tile as tile
from concourse import mybir
from concourse.bass_utils import run_bass_kernel_spmd

BF16 = mybir.dt.bfloat16
F32 = mybir.dt.float32
AF = mybir.ActivationFunctionType
NPBF = ml_dtypes.bfloat16

N = 2048          # sequence length
EMB = 1024        # model dim
HL = 4            # heads per core
D = 64            # head dim
EC = HL * D       # head dims per core (256)
KT = N // 128     # 16 k-token tiles
QW = 1024         # q chunk width
NQH = N // QW     # 2 q chunks
SCALE = 1.0 / np.sqrt(D)

_CACHE = {}


def build_program():
    nc = bacc.Bacc("TRN2", target_bir_lowering=False, debug=False, num_devices=8)

    xq_d = nc.declare_dram_parameter("xqT", [EMB, N], BF16, isOutput=False)
    xk_d = nc.declare_dram_parameter("xkT", [EMB, N], BF16, isOutput=False)
    xv_d = nc.declare_dram_parameter("xvT", [EMB, N], BF16, isOutput=False)
    wq_d = nc.declare_dram_parameter("wq", [EMB, EC], BF16, isOutput=False)
    wk_d = nc.declare_dram_parameter("wk", [EMB, EC], BF16, isOutput=False)
    wv_d = nc.declare_dram_parameter("wv", [EMB, EC], BF16, isOutput=False)
    wo_d = nc.declare_dram_parameter("wo", [EC, EMB], BF16, isOutput=False)
    bqf_d = nc.declare_dram_parameter("bqf", [128, 2], F32, isOutput=False)
    bkf_d = nc.declare_dram_parameter("bkf", [128, 2], F32, isOutput=False)
    bv_d = nc.declare_dram_parameter("bv", [1, EC], BF16, isOutput=False)
    out_d = nc.declare_dram_parameter("outT", [EMB, N], BF16, isOutput=True)
    tw_d = [nc.declare_dram_parameter(f"tw{h}", [N, N], BF16, isOutput=True)
            for h in range(HL)]

    with ExitStack() as ctx:
        tc = ctx.enter_context(tile.TileContext(nc))

        const = ctx.enter_context(tc.tile_pool(name="const", bufs=1))
        ones_bf = const.tile([1, 512], BF16, name="ones_bf")
        nc.vector.memset(ones_bf[:], 1.0)
        bqf_sb = const.tile([128, 2], F32, name="bqf_sb")
        bkf_sb = const.tile([128, 2], F32, name="bkf_sb")
        bv_sb = const.tile([1, EC], BF16, name="bv_sb")
        nc.sync.dma_start(bqf_sb[:], bqf_d[:])
        nc.sync.dma_start(bkf_sb[:], bkf_d[:])
        nc.sync.dma_start(bv_sb[:], bv_d[:])

        wpool = ctx.enter_context(tc.tile_pool(name="wpool", bufs=1))
        wq_sb = wpool.tile([128, 8, EC], BF16, name="wq_sb")  # k-chunk major
        wk_sb = wpool.tile([128, 8, EC], BF16, name="wk_sb")
        wv_sb = wpool.tile([128, 8, EC], BF16, name="wv_sb")
        wo_sb = wpool.tile([128, 2, EMB], BF16, name="wo_sb")
        for kc in range(8):
            nc.sync.dma_start(wk_sb[:, kc, :], wk_d[kc * 128:(kc + 1) * 128, :])
            nc.sync.dma_start(wq_sb[:, kc, :], wq_d[kc * 128:(kc + 1) * 128, :])
            nc.sync.dma_start(wv_sb[:, kc, :], wv_d[kc * 128:(kc + 1) * 128, :])
        for hc in range(2):
            nc.sync.dma_start(wo_sb[:, hc, :], wo_d[hc * 128:(hc + 1) * 128, :])

        qkv = ctx.enter_context(tc.tile_pool(name="qkv", bufs=1))
        qhT = qkv.tile([128, 2, N], BF16, name="qhT")      # [dim-half][128, tok]
        khT = qkv.tile([128, 2, N], BF16, name="khT")
        vh = qkv.tile([128, KT, HL, 65], BF16, name="vh")  # [tok128][kt][h][64+ones]
        an = qkv.tile([128, 2, N], BF16, name="an")        # A_norm^T [hd-chunk][q]
        nc.vector.memset(vh[:, :, :, 64:65], 1.0)

        # ---------------- phase A: projections ----------------
        with tc.tile_pool(name="xpool", bufs=24) as xpool, \
             tc.tile_pool(name="prps", bufs=4, space="PSUM") as prps:

            def load_x(x_d, pfx):
                xt = []
                for kc in range(8):
                    xtile = xpool.tile([128, N], BF16, name=f"{pfx}{kc}", tag="x")
                    nc.sync.dma_start(xtile[:], x_d[kc * 128:(kc + 1) * 128, :])
                    xt.append(xtile)
                return xt

            xk_t = load_x(xk_d, "xk")
            xq_t = load_x(xq_d, "xq")
            xv_t = load_x(xv_d, "xv")

            def proj_qk_group(xt, w_sb, bf_sb, dst, dh, q4):
                ps = prps.tile([128, 512], F32, name="pj", tag="prps")
                for kc in range(8):
                    nc.tensor.matmul(
                        ps[:], w_sb[:, kc, dh * 128:(dh + 1) * 128],
                        xt[kc][:, q4 * 512:(q4 + 1) * 512],
                        start=(kc == 0), stop=(kc == 7))
                nc.scalar.activation(dst[:, dh, q4 * 512:(q4 + 1) * 512], ps[:],
                                     AF.Identity, bias=bf_sb[:, dh:dh + 1],
                                     scale=1.0)

            def proj_v_group(t):
                ps = prps.tile([128, EC], F32, name="pv", tag="prps")
                for kc in range(8):
                    nc.tensor.matmul(ps[:], xv_t[kc][:, t * 128:(t + 1) * 128],
                                     wv_sb[:, kc, :], start=(kc == 0), stop=False)
                nc.tensor.matmul(ps[:], ones_bf[0:1, 0:128], bv_sb[0:1, :],
                                 start=False, stop=True)
                nc.scalar.copy(vh[:, t, :, 0:64],
                               ps[:].rearrange("p (h c) -> p h c", h=HL))

            for q4 in range(4):
                proj_qk_group(xk_t, wk_sb, bkf_sb, khT, 0, q4)
            for q4 in range(2):
                proj_qk_group(xq_t, wq_sb, bqf_sb, qhT, 0, q4)
            for t in range(KT):
                proj_v_group(t)
            for q4 in range(4):
                proj_qk_group(xk_t, wk_sb, bkf_sb, khT, 1, q4)
            for q4 in range(2, 4):
                proj_qk_group(xq_t, wq_sb, bqf_sb, qhT, 0, q4)
            for q4 in range(4):
                proj_qk_group(xq_t, wq_sb, bqf_sb, qhT, 1, q4)

        # ---------------- phase B: attention ----------------
        scps = ctx.enter_context(tc.tile_pool(name="scps", bufs=2, space="PSUM"))
        avps = ctx.enter_context(tc.tile_pool(name="avps", bufs=1, space="PSUM"))
        epool = ctx.enter_context(tc.tile_pool(name="epool", bufs=38))
        evp = ctx.enter_context(tc.tile_pool(name="evp", bufs=3))
        rbp = ctx.enter_context(tc.tile_pool(name="rbp", bufs=3))
        twp = ctx.enter_context(tc.tile_pool(name="twp", bufs=4))
        otp = ctx.enter_context(tc.tile_pool(name="otp", bufs=3))
        smol = ctx.enter_context(tc.tile_pool(name="smol", bufs=2))

        def outproj_group(qc, m, qn):
            """Output projection for one [128, 512] tile of out^T."""
            q0 = qc * QW + qn * 512
            ps = scps.tile([128, 512], F32, name="op", tag="ps")
            for hc in range(2):
                nc.tensor.matmul(ps[:], wo_sb[:, hc, m * 128:(m + 1) * 128],
                                 an[:, hc, q0:q0 + 512],
                                 start=(hc == 0), stop=(hc == 1))
            ot = otp.tile([128, 512], BF16, name="ot", tag="ot")
            if (m + qn) % 2:
                nc.scalar.copy(ot[:], ps[:])
            else:
                nc.vector.tensor_copy(ot[:], ps[:])
            nc.gpsimd.dma_start(
                out_d[m * 128:(m + 1) * 128, q0:q0 + 512], ot[:])

        def make_norm_block(hp, q0, evA, evB, rA, rB, EA, EB):
            hA, hB = 2 * hp, 2 * hp + 1

            def norm_block():
                # broadcast 1/rowsum to 128 partitions via K=1 matmuls
                rbAp = scps.tile([128, QW], F32, name="rbAp", tag="ps")
                rbBp = scps.tile([128, QW], F32, name="rbBp", tag="ps")
                for nx in (0, 512):
                    nc.tensor.matmul(rbAp[:, nx:nx + 512], ones_bf[0:1, 0:128],
                                     rA[0:1, nx:nx + 512], start=True, stop=True)
                for nx in (0, 512):
                    nc.tensor.matmul(rbBp[:, nx:nx + 512], ones_bf[0:1, 0:128],
                                     rB[0:1, nx:nx + 512], start=True, stop=True)
                rbA = rbp.tile([128, QW], BF16, name="rbA", tag="rb")
                rbB = rbp.tile([128, QW], BF16, name="rbB", tag="rb")
                nc.vector.tensor_copy(rbA[:], rbAp[:])
                nc.vector.tensor_copy(rbB[:], rbBp[:])
                # normalized attention outputs (A_norm^T)
                nc.vector.tensor_mul(an[0:64, hp, q0:q0 + QW],
                                     evA[0:64, :], rbA[0:64, :])
                tmpB = twp.tile([64, QW], BF16, name="tmpB", tag="tw")
                nc.vector.tensor_mul(tmpB[:], evB[0:64, :], rbB[0:64, :])
                nc.gpsimd.dma_start(an[64:128, hp, q0:q0 + QW], tmpB[:])
                # tw partials: normalized per-head prob maps, streamed out
                for kt in range(KT):
                    twA = twp.tile([128, QW], BF16, name="twA", tag="tw")
                    twB = twp.tile([128, QW], BF16, name="twB", tag="tw")
                    nc.vector.tensor_mul(twA[:], EA[kt][:], rbA[:])
                    nc.vector.tensor_mul(twB[:], EB[kt][:], rbB[:])
                    nc.gpsimd.dma_start(
                        tw_d[hA][kt * 128:(kt + 1) * 128, q0:q0 + QW], twA[:])
                    nc.gpsimd.dma_start(
                        tw_d[hB][kt * 128:(kt + 1) * 128, q0:q0 + QW], twB[:])

            return norm_block

        pending_norm = None
        for qc in range(NQH):
            q0 = qc * QW
            for hp in range(2):
                hA, hB = 2 * hp, 2 * hp + 1
                avA = avps.tile([65, QW], F32, name="avA", tag="avA")
                avB = avps.tile([65, QW], F32, name="avB", tag="avB")
                EA, EB = [], []
                for kt in range(KT):
                    psA = scps.tile([128, QW], F32, name="psA", tag="ps")
                    psB = scps.tile([128, QW], F32, name="psB", tag="ps")
                    kcols = slice(kt * 128, (kt + 1) * 128)
                    # A/B interleave: consecutive MMs alternate PE row groups
                    # and weight slots, so each LDWEIGHTS hides under the
                    # other head's matmul stream.
                    for nx in (0, 512):
                        nc.tensor.matmul(psA[:, nx:nx + 512],
                                         khT[0:64, hp, kcols],
                                         qhT[0:64, hp, q0 + nx:q0 + nx + 512],
                                         start=True, stop=True)
                        nc.tensor.matmul(psB[:, nx:nx + 512],
                                         khT[64:128, hp, kcols],
                                         qhT[64:128, hp, q0 + nx:q0 + nx + 512],
                                         start=True, stop=True)
                    eA = epool.tile([128, QW], BF16, name="eA", tag="E")
                    eB = epool.tile([128, QW], BF16, name="eB", tag="E")
                    nc.scalar.activation(eA[:], psA[:], AF.Exp, scale=float(SCALE))
                    nc.scalar.activation(eB[:], psB[:], AF.Exp, scale=float(SCALE))
                    EA.append(eA)
                    EB.append(eB)
                    for nx in (0, 512):
                        nc.tensor.matmul(avA[:, nx:nx + 512], vh[:, kt, hA, :],
                                         eA[:, nx:nx + 512],
                                         start=(kt == 0), stop=(kt == KT - 1))
                        nc.tensor.matmul(avB[:, nx:nx + 512], vh[:, kt, hB, :],
                                         eB[:, nx:nx + 512],
                                         start=(kt == 0), stop=(kt == KT - 1))
                    if kt == 2 and pending_norm is not None:
                        pending_norm()
                        pending_norm = None
                # boundary: evict the accumulators (freeing their PSUM banks,
                # exposing rowsum rows) and run the packed-reciprocal chain.
                evA = evp.tile([65, QW], BF16, name="evA", tag="ev")
                evB = evp.tile([65, QW], BF16, name="evB", tag="ev")
                nc.vector.tensor_copy(evA[:], avA[:])
                nc.scalar.copy(evB[:], avB[:])
                rsp = smol.tile([128, 16], BF16, name="rsp", tag="rsp")
                nc.sync.dma_start(rsp[0:64, :], evA[64:65, :])
                nc.sync.dma_start(rsp[64:128, :], evB[64:65, :])
                rcp = smol.tile([128, 16], F32, name="rcp", tag="rcp")
                nc.vector.reciprocal(rcp[:], rsp[:])
                rcb = smol.tile([128, 16], BF16, name="rcb", tag="rcb")
                nc.vector.tensor_copy(rcb[:], rcp[:])
                rA = smol.tile([1, QW], BF16, name="rA", tag="rA")
                rB = smol.tile([1, QW], BF16, name="rB", tag="rB")
                nc.sync.dma_start(rA[:], rcb[0:64, :])
                nc.sync.dma_start(rB[:], rcb[64:128, :])
                pending_norm = make_norm_block(hp, q0, evA, evB, rA, rB, EA, EB)

        # ---------------- tail: output projections + last norm ----------------
        for m in range(8):
            for qn in range(2):
                outproj_group(0, m, qn)
        pending_norm()
        for m in range(8):
            for qn in range(2):
                outproj_group(1, m, qn)

    nc.compile()
    return nc


def _get_program():
    if "nc" not in _CACHE:
        _CACHE["nc"] = build_program()
    return _CACHE["nc"]


def _bf(x):
    return np.ascontiguousarray(np.asarray(x, dtype=np.float32)).astype(NPBF)


def make_in_maps(q, k, v, Wq, bq, Wk, bk, Wv, bv, Wo):
    in_maps = []
    for c in range(8):
        b, hg = c // 4, c % 4
        sl = slice(hg * EC, (hg + 1) * EC)
        in_maps.append({
            "xqT": _bf(q[b].T), "xkT": _bf(k[b].T), "xvT": _bf(v[b].T),
            "wq": _bf(Wq[:, sl]), "wk": _bf(Wk[:, sl]), "wv": _bf(Wv[:, sl]),
            "wo": _bf(Wo[sl, :]),
            "bqf": np.ascontiguousarray(
                np.asarray(bq[sl], np.float32).reshape(2, 128).T),
            "bkf": np.ascontiguousarray(
                np.asarray(bk[sl], np.float32).reshape(2, 128).T),
            "bv": _bf(bv[sl].reshape(1, EC)),
        })
    return in_maps


def gather_results(results, bo):
    out = np.zeros((2, N, EMB), np.float32)
    tw = np.zeros((N, N), np.float32)
    for c in range(8):
        b = c // 4
        out[b] += results[c]["outT"].astype(np.float32).T
        if b == 0:
            for h in range(HL):
                tw += results[c][f"tw{h}"].astype(np.float32)
    out += np.asarray(bo, dtype=np.float32).reshape(1, 1, EMB)
    return out, np.ascontiguousarray(tw.T)


def kernel(q, k, v, Wq, bq, Wk, bk, Wv, bv, Wo, bo, **run_kwargs):
    nc = _get_program()
    in_maps = make_in_maps(q, k, v, Wq, bq, Wk, bk, Wv, bv, Wo)
    res = run_bass_kernel_spmd(nc, in_maps, list(range(8)), **run_kwargs)
    out, tw = gather_results(res.results, bo)
    if run_kwargs:
        return (out, tw), res
    return (out, tw)
